# revision 18
# baseline (speedup 1.0000x reference)
"""Trainium2 Bass kernel for nn_Attention_61830349193262.

Math per batch b (S = T = 2048, D = 1024):
    scores[s,t] = <state[s,:], x[t,:]>            (masked rows s where src==0)
    p_attn      = softmax_s(scores)               -> [S,T]
    w[t,d]      = sum_s state[s,d] p_attn[s,t]    (rows t where src==0 -> -inf)
    attn        = softmax_t(w)                    -> [T,D]
    out[e,d]    = sum_t state[t,d] attn[t,e]      -> [D,D]

Sharding: data-parallel over batch, one batch per NeuronCore (8 cores).

Device pipeline (per core):
  - All matmul operands are fp16 (full PE rate on trn2, ~8x finer mantissa
    than bf16); PSUM accumulation and softmax statistics are fp32.
  - Masking is multiplicative *after* exp (exp row-max may include masked
    entries; softmax is shift-invariant so this is exact), which avoids any
    additive -1e9 bias matmuls.
  - All transposes run on the DMA xbar (2-byte dtype), not the PE:
      a [128, F] -> 3D [128, F/128, 128] transpose writes logical row r of
      the transposed matrix to (p = r % 128, c = r // 128), i.e. out[:, c, :]
      is the natural 128-row chunk c of the transposed matrix. Stationary
      operand chunks therefore pair with plain natural state chunks.
"""

import os
import numpy as np

_PHASES = int(os.environ.get("K_PHASES", "9"))  # debug bisect: 0=setup,1=+1a,2=+1b,9=full

B, S, D = 8, 2048, 1024
NT = S // 128       # 16 t-blocks
NS = S // 128       # 16 s-chunks
ND = D // 128       # 8 d-chunks
NE = D // 128       # 8 e-chunks
TSUP = 512          # t-superblock for phase 1b
NSUP = S // TSUP    # 4

_CACHED = {}


def _build():
    import concourse.bass as bass
    import concourse.mybir as mybir
    import concourse.tile as tile
    from concourse import bacc

    f32 = mybir.dt.float32
    f16 = mybir.dt.float16
    Alu = mybir.AluOpType
    Act = mybir.ActivationFunctionType
    Ax = mybir.AxisListType

    nc = bacc.Bacc("TRN2", target_bir_lowering=False, debug=False, num_devices=8)

    state_d = nc.dram_tensor("state", [S, D], f32, kind="ExternalInput").ap()
    x_d = nc.dram_tensor("x", [S, D], f32, kind="ExternalInput").ap()
    keep_d = nc.dram_tensor("keep", [S], f16, kind="ExternalInput").ap()
    out_d = nc.dram_tensor("out", [D, D], f32, kind="ExternalOutput").ap()

    with tile.TileContext(nc) as tc:
        with (
            tc.tile_pool(name="persist", bufs=1) as persist,
            tc.tile_pool(name="stage", bufs=2) as stage,
            tc.tile_pool(name="etr", bufs=2) as etrp,
            tc.tile_pool(name="work", bufs=2) as work,
            tc.tile_pool(name="sms", bufs=2) as smsp,
            tc.tile_pool(name="small", bufs=2) as small,
            tc.tile_pool(name="stats", bufs=24) as stats,
            tc.tile_pool(name="osb", bufs=2) as osb,
            tc.tile_pool(name="ps_s", bufs=5, space="PSUM") as ps_s,
            tc.tile_pool(name="ps_w", bufs=2, space="PSUM") as ps_w,
            tc.tile_pool(name="ps_o", bufs=1, space="PSUM") as ps_o,
        ):
            # ---- constants / persistent inputs ----
            keep_bc = persist.tile([128, S], f16)
            keep_b = bass.AP(
                tensor=keep_d.tensor,
                offset=keep_d.offset,
                ap=[[0, 128]] + list(keep_d.ap),
            )
            nc.gpsimd.dma_start(out=keep_bc[:], in_=keep_b)

            # state in natural s-chunks: state_sig[p, c, d] = state[128*c + p, d]
            # (the xbar 3D-out transpose produces chunks in this same natural
            # blocked order: out[p, c, f] = in^T[128*c + p, f])
            state_sig = persist.tile([128, 16, D], f16)
            for c in range(16):
                nc.gpsimd.dma_start(
                    out=state_sig[:, c, :], in_=state_d[c * 128 : (c + 1) * 128, :]
                )

            # stateT: state_tr[p2, dc, s] = state[s, 128*dc + p2]
            state_tr = persist.tile([128, ND, S], f16)
            for sc in range(NS):
                nc.sync.dma_start(
                    out=state_tr[:, :, sc * 128 : (sc + 1) * 128],
                    in_=state_sig[:, sc, :],
                    transpose=True,
                )

            # wT[d, t] staged as wt_big[pd, dc, t] = w[128*dc + pd, t]
            wt_big = persist.tile([128, ND, S], f16)

            if _PHASES == 0:
                dummy = osb.tile([128, D], f32, tag="out_sb")
                nc.vector.tensor_copy(dummy[:, 0:16], state_sig[:, 0, 0:16])
                nc.vector.tensor_copy(dummy[:, 16:32], state_tr[:, 0, 0:16])
                nc.sync.dma_start(out=out_d[0:128, :], in_=dummy[:])

            # ---- phase 1: scores softmax -> E, then wT = state^T @ E^T ----
            for ts in range(NSUP if _PHASES >= 1 else 0):
                etr = etrp.tile([128, 16, TSUP], f16, tag="etr")
                for tbl in range(NSUP):
                    tb = ts * NSUP + tbl
                    # xT tile for this t-block: x_tr[p2, dc, t'] = x[t0+t', 128*dc+p2]
                    x_nat = stage.tile([128, D], f16, tag="x_nat")
                    nc.gpsimd.dma_start(
                        out=x_nat[:], in_=x_d[tb * 128 : (tb + 1) * 128, :]
                    )
                    x_tr = stage.tile([128, ND, 128], f16, tag="x_tr")
                    nc.sync.dma_start(out=x_tr[:], in_=x_nat[:], transpose=True)

                    # scoresT[t', s] in 4 psum quarters of [128, 512]
                    quarters = []
                    for q in range(4):
                        psq = ps_s.tile([128, 512], f32, tag="psq")
                        for dc in range(ND):
                            nc.tensor.matmul(
                                psq[:],
                                x_tr[:, dc, :],
                                state_tr[:, dc, q * 512 : (q + 1) * 512],
                                start=(dc == 0),
                                stop=(dc == ND - 1),
                            )
                        quarters.append(psq)

                    # Mask before the row-max: sms = (score + 60000) * keep.
                    # Masked columns become exactly 0; unmasked ~60000+score,
                    # so the max always comes from an unmasked column and
                    # exp(0 - max) underflows to exactly 0 for masked ones.
                    sms = smsp.tile([128, S], f32, tag="sms")
                    for q in range(4):
                        nc.vector.scalar_tensor_tensor(
                            out=sms[:, q * 512 : (q + 1) * 512],
                            in0=quarters[q][:],
                            scalar=60000.0,
                            in1=keep_bc[:, q * 512 : (q + 1) * 512],
                            op0=Alu.add,
                            op1=Alu.mult,
                        )
                    nmax = stats.tile([128, 1], f32, tag="nmax")
                    nc.vector.reduce_max(nmax[:], sms[:], axis=Ax.X, negate=True)

                    e_raw = work.tile([128, S], f16, tag="e_raw")
                    zsum = stats.tile([128, 1], f32, tag="zsum")
                    nc.scalar.activation(
                        e_raw[:],
                        sms[:],
                        Act.Exp,
                        bias=nmax[:],
                        scale=1.0,
                        accum_out=zsum[:],
                    )
                    rz = stats.tile([128, 1], f32, tag="rz")
                    nc.vector.reciprocal(rz[:], zsum[:])
                    e_n = work.tile([128, S], f16, tag="e_n")
                    nc.vector.tensor_scalar_mul(e_n[:], e_raw[:], rz[:])

                    # E^T into etr: etr[p3, c3, tbl*128 + t'] = e_n[t', 128*c3 + p3]
                    nc.sync.dma_start(
                        out=etr[:, :, tbl * 128 : (tbl + 1) * 128],
                        in_=e_n[:],
                        transpose=True,
                    )

                # phase 1b for this superblock: wT[d, t] += state[s, d]^T E^T[s, t]
                for dc in range(ND if _PHASES >= 2 else 0):
                    pw = ps_w.tile([128, TSUP], f32, tag="pw")
                    for c3 in range(16):
                        nc.tensor.matmul(
                            pw[:],
                            state_sig[:, c3, dc * 128 : (dc + 1) * 128],
                            etr[:, c3, :],
                            start=(c3 == 0),
                            stop=(c3 == 15),
                        )
                    nc.vector.tensor_copy(
                        wt_big[:, dc, ts * TSUP : (ts + 1) * TSUP], pw[:]
                    )

            # ---- phase 2: softmax over t of wT rows, then out = attn^T @ state ----
            for ec in range(NE if _PHASES >= 3 else 0):
                wrow = wt_big[:, ec, :]  # [128, 2048] fp16, e = 128*ec + p
                nmax2 = stats.tile([128, 1], f32, tag="nmax2")
                nc.vector.reduce_max(nmax2[:], wrow, axis=Ax.X, negate=True)
                a_raw = work.tile([128, S], f16, tag="e_raw")
                nc.scalar.activation(a_raw[:], wrow, Act.Exp, bias=nmax2[:], scale=1.0)
                a_m = work.tile([128, S], f16, tag="e_m")
                nc.vector.tensor_mul(a_m[:], a_raw[:], keep_bc[:])
                z2 = stats.tile([128, 1], f32, tag="z2")
                nc.vector.reduce_sum(z2[:], a_m[:], axis=Ax.X)
                rz2 = stats.tile([128, 1], f32, tag="rz2")
                nc.vector.reciprocal(rz2[:], z2[:])
                a_n = work.tile([128, S], f16, tag="e_n")
                nc.vector.tensor_scalar_mul(a_n[:], a_m[:], rz2[:])

                a_tr = small.tile([128, 16, 128], f16, tag="a_tr")
                nc.sync.dma_start(out=a_tr[:], in_=a_n[:], transpose=True)

                out_sb = osb.tile([128, D], f32, tag="out_sb")
                for dh in range(2):
                    po = ps_o.tile([128, 512], f32, tag="po")
                    for c4 in range(16):
                        nc.tensor.matmul(
                            po[:],
                            a_tr[:, c4, :],
                            state_sig[:, c4, dh * 512 : (dh + 1) * 512],
                            start=(c4 == 0),
                            stop=(c4 == 15),
                        )
                    nc.vector.tensor_copy(out_sb[:, dh * 512 : (dh + 1) * 512], po[:])
                nc.sync.dma_start(
                    out=out_d[ec * 128 : (ec + 1) * 128, :], in_=out_sb[:]
                )

    nc.compile()
    return nc


def get_nc():
    if "nc" not in _CACHED:
        _CACHED["nc"] = _build()
    return _CACHED["nc"]


def _make_in_maps(state, x, src):
    state = np.ascontiguousarray(np.asarray(state, dtype=np.float32))
    x = np.ascontiguousarray(np.asarray(x, dtype=np.float32))
    src = np.asarray(src)
    keep = (src != 0).astype(np.float16)
    return [{"state": state[b], "x": x[b], "keep": keep[b]} for b in range(B)]


def run_bass(state, x, src, trace=False, **trace_kwargs):
    from concourse.bass_utils import run_bass_kernel_spmd

    nc = get_nc()
    in_maps = _make_in_maps(state, x, src)
    res = run_bass_kernel_spmd(
        nc, in_maps, core_ids=list(range(B)), trace=trace, **trace_kwargs
    )
    out = np.stack([res.results[b]["out"] for b in range(B)]).astype(np.float32)
    return out, res


def kernel(state, x, src, **kwargs):
    out, _ = run_bass(state, x, src, trace=False)
    return out


if __name__ == "__main__":
    rng = np.random.default_rng(0)
    st = rng.standard_normal((B, S, D), dtype=np.float32)
    xx = rng.standard_normal((B, S, D), dtype=np.float32)
    sr = rng.integers(0, 5, size=(B, S))
    o = kernel(state=st, x=xx, src=sr)
    print(o.shape, o.dtype, np.abs(o).max())


# revision 25
# speedup vs baseline: 1.1827x; 1.1827x over previous
"""Trainium2 Bass kernel for nn_Attention_61830349193262.

Math per batch b (S = T = 2048, D = 1024):
    scores[s,t] = <state[s,:], x[t,:]>            (masked rows s where src==0)
    p_attn      = softmax_s(scores)               -> [S,T]
    w[t,d]      = sum_s state[s,d] p_attn[s,t]    (rows t where src==0 -> -inf)
    attn        = softmax_t(w)                    -> [T,D]
    out[e,d]    = sum_t state[t,d] attn[t,e]      -> [D,D]

Sharding: data-parallel over batch, one batch per NeuronCore (8 cores).

Device pipeline (per core):
  - All matmul operands are fp16 (full PE rate on trn2, ~8x finer mantissa
    than bf16); PSUM accumulation and softmax statistics are fp32.
  - Masking is multiplicative *after* exp (exp row-max may include masked
    entries; softmax is shift-invariant so this is exact), which avoids any
    additive -1e9 bias matmuls.
  - All transposes run on the DMA xbar (2-byte dtype), not the PE:
      a [128, F] -> 3D [128, F/128, 128] transpose writes logical row r of
      the transposed matrix to (p = r % 128, c = r // 128), i.e. out[:, c, :]
      is the natural 128-row chunk c of the transposed matrix. Stationary
      operand chunks therefore pair with plain natural state chunks.
"""

import os
import numpy as np

_PHASES = int(os.environ.get("K_PHASES", "9"))  # debug bisect: 0=setup,1=+1a,2=+1b,9=full

B, S, D = 8, 2048, 1024
NT = S // 128       # 16 t-blocks
NS = S // 128       # 16 s-chunks
ND = D // 128       # 8 d-chunks
NE = D // 128       # 8 e-chunks
TSUP = 512          # t-superblock for phase 1b
NSUP = S // TSUP    # 4

_CACHED = {}


def _build():
    import concourse.bass as bass
    import concourse.mybir as mybir
    import concourse.tile as tile
    from concourse import bacc

    f32 = mybir.dt.float32
    f16 = mybir.dt.float16
    Alu = mybir.AluOpType
    Act = mybir.ActivationFunctionType
    Ax = mybir.AxisListType

    nc = bacc.Bacc("TRN2", target_bir_lowering=False, debug=False, num_devices=8)

    state_d = nc.dram_tensor("state", [S, D], f32, kind="ExternalInput").ap()
    x_d = nc.dram_tensor("x", [S, D], f32, kind="ExternalInput").ap()
    keep_d = nc.dram_tensor("keep", [S], f16, kind="ExternalInput").ap()
    out_d = nc.dram_tensor("out", [D, D], f32, kind="ExternalOutput").ap()

    with tile.TileContext(nc) as tc:
        with (
            tc.tile_pool(name="persist", bufs=1) as persist,
            tc.tile_pool(name="stage", bufs=3) as stage,
            tc.tile_pool(name="etr", bufs=2) as etrp,
            tc.tile_pool(name="work", bufs=2) as work,
            tc.tile_pool(name="sms", bufs=2) as smsp,
            tc.tile_pool(name="small", bufs=2) as small,
            tc.tile_pool(name="stats", bufs=12) as stats,
            tc.tile_pool(name="osb", bufs=2) as osb,
            tc.tile_pool(name="ps_s", bufs=6, space="PSUM") as ps_s,
            tc.tile_pool(name="ps_w", bufs=2, space="PSUM") as ps_w,
        ):
            # ---- constants / persistent inputs ----
            keep_bc = persist.tile([128, S], f16)
            keep_b = bass.AP(
                tensor=keep_d.tensor,
                offset=keep_d.offset,
                ap=[[0, 128]] + list(keep_d.ap),
            )
            nc.gpsimd.dma_start(out=keep_bc[:], in_=keep_b)

            # state in natural s-chunks: state_sig[p, c, d] = state[128*c + p, d]
            # (the xbar 3D-out transpose produces chunks in this same natural
            # blocked order: out[p, c, f] = in^T[128*c + p, f]).
            # Load fp32 via HWDGE and cast on DVE — SWDGE casting DMA runs at
            # ~100 GB/s and serializes the whole startup.
            state_sig = persist.tile([128, 16, D], f16)
            for c in range(16):
                st_f32 = stage.tile([128, D], f32, tag="f32stage")
                nc.sync.dma_start(
                    out=st_f32[:], in_=state_d[c * 128 : (c + 1) * 128, :]
                )
                nc.vector.tensor_copy(state_sig[:, c, :], st_f32[:])

            # stateT: state_tr[p2, dc, s] = state[s, 128*dc + p2]
            state_tr = persist.tile([128, ND, S], f16)
            for sc in range(NS):
                nc.sync.dma_start(
                    out=state_tr[:, :, sc * 128 : (sc + 1) * 128],
                    in_=state_sig[:, sc, :],
                    transpose=True,
                )

            # wT[d, t] staged as wt_big[pd, dc, t] = w[128*dc + pd, t]
            wt_big = persist.tile([128, ND, S], f16)

            if _PHASES == 0:
                dummy = osb.tile([128, D], f32, tag="out_sb")
                nc.vector.tensor_copy(dummy[:, 0:16], state_sig[:, 0, 0:16])
                nc.vector.tensor_copy(dummy[:, 16:32], state_tr[:, 0, 0:16])
                nc.sync.dma_start(out=out_d[0:128, :], in_=dummy[:])

            # ---- phase 1: scores softmax -> E, then wT = state^T @ E^T ----
            for ts in range(NSUP if _PHASES >= 1 else 0):
                etr = etrp.tile([128, 16, TSUP], f16, tag="etr")
                for tbl in range(NSUP):
                    tb = ts * NSUP + tbl
                    # xT tile for this t-block: x_tr[p2, dc, t'] = x[t0+t', 128*dc+p2]
                    x_f32 = stage.tile([128, D], f32, tag="f32stage")
                    nc.sync.dma_start(
                        out=x_f32[:], in_=x_d[tb * 128 : (tb + 1) * 128, :]
                    )
                    x_nat = stage.tile([128, D], f16, tag="x_nat")
                    nc.scalar.copy(x_nat[:], x_f32[:])
                    x_tr = stage.tile([128, ND, 128], f16, tag="x_tr")
                    nc.sync.dma_start(out=x_tr[:], in_=x_nat[:], transpose=True)

                    # scoresT[t', s] in 4 psum quarters of [128, 512]
                    quarters = []
                    for q in range(4):
                        psq = ps_s.tile([128, 512], f32, tag="psq")
                        for dc in range(ND):
                            nc.tensor.matmul(
                                psq[:],
                                x_tr[:, dc, :],
                                state_tr[:, dc, q * 512 : (q + 1) * 512],
                                start=(dc == 0),
                                stop=(dc == ND - 1),
                            )
                        quarters.append(psq)

                    # Mask before the row-max: sms = (score + 60000) * keep.
                    # Masked columns become exactly 0; unmasked ~60000+score,
                    # so the max always comes from an unmasked column and
                    # exp(0 - max) underflows to exactly 0 for masked ones.
                    sms = smsp.tile([128, S], f32, tag="sms")
                    for q in range(4):
                        nc.vector.scalar_tensor_tensor(
                            out=sms[:, q * 512 : (q + 1) * 512],
                            in0=quarters[q][:],
                            scalar=60000.0,
                            in1=keep_bc[:, q * 512 : (q + 1) * 512],
                            op0=Alu.add,
                            op1=Alu.mult,
                        )
                    nmax = stats.tile([128, 1], f32, tag="nmax")
                    nc.vector.reduce_max(nmax[:], sms[:], axis=Ax.X, negate=True)

                    e_raw = work.tile([128, S], f16, tag="e_raw")
                    zsum = stats.tile([128, 1], f32, tag="zsum")
                    nc.scalar.activation(
                        e_raw[:],
                        sms[:],
                        Act.Exp,
                        bias=nmax[:],
                        scale=1.0,
                        accum_out=zsum[:],
                    )
                    rz = stats.tile([128, 1], f32, tag="rz")
                    nc.vector.reciprocal(rz[:], zsum[:])
                    e_n = work.tile([128, S], f16, tag="e_n")
                    nc.vector.tensor_scalar_mul(e_n[:], e_raw[:], rz[:])

                    # E^T into etr: etr[p3, c3, tbl*128 + t'] = e_n[t', 128*c3 + p3]
                    nc.sync.dma_start(
                        out=etr[:, :, tbl * 128 : (tbl + 1) * 128],
                        in_=e_n[:],
                        transpose=True,
                    )

                # phase 1b for this superblock: wT[d, t] += state[s, d]^T E^T[s, t]
                for dc in range(ND if _PHASES >= 2 else 0):
                    pw = ps_w.tile([128, TSUP], f32, tag="pw")
                    for c3 in range(16):
                        nc.tensor.matmul(
                            pw[:],
                            state_sig[:, c3, dc * 128 : (dc + 1) * 128],
                            etr[:, c3, :],
                            start=(c3 == 0),
                            stop=(c3 == 15),
                        )
                    nc.vector.tensor_copy(
                        wt_big[:, dc, ts * TSUP : (ts + 1) * TSUP], pw[:]
                    )

            # ---- phase 2: softmax over t of wT rows, then out = attn^T @ state ----
            for ec in range(NE if _PHASES >= 3 else 0):
                wrow = wt_big[:, ec, :]  # [128, 2048] fp16, e = 128*ec + p
                nmax2 = stats.tile([128, 1], f32, tag="nmax2")
                nc.vector.reduce_max(nmax2[:], wrow, axis=Ax.X, negate=True)
                a_raw = work.tile([128, S], f16, tag="e_raw")
                nc.scalar.activation(a_raw[:], wrow, Act.Exp, bias=nmax2[:], scale=1.0)
                a_m = smsp.tile([128, S], f16, tag="sms")
                nc.vector.tensor_mul(a_m[:], a_raw[:], keep_bc[:])
                z2 = stats.tile([128, 1], f32, tag="z2")
                nc.vector.reduce_sum(z2[:], a_m[:], axis=Ax.X)
                rz2 = stats.tile([128, 1], f32, tag="rz2")
                nc.vector.reciprocal(rz2[:], z2[:])
                a_n = work.tile([128, S], f16, tag="e_n")
                nc.vector.tensor_scalar_mul(a_n[:], a_m[:], rz2[:])

                a_tr = small.tile([128, 16, 128], f16, tag="a_tr")
                nc.sync.dma_start(out=a_tr[:], in_=a_n[:], transpose=True)

                out_sb = osb.tile([128, D], f32, tag="out_sb")
                for dh in range(2):
                    po = ps_w.tile([128, 512], f32, tag="pw")
                    for c4 in range(16):
                        nc.tensor.matmul(
                            po[:],
                            a_tr[:, c4, :],
                            state_sig[:, c4, dh * 512 : (dh + 1) * 512],
                            start=(c4 == 0),
                            stop=(c4 == 15),
                        )
                    nc.vector.tensor_copy(out_sb[:, dh * 512 : (dh + 1) * 512], po[:])
                nc.sync.dma_start(
                    out=out_d[ec * 128 : (ec + 1) * 128, :], in_=out_sb[:]
                )

    nc.compile()
    return nc


def get_nc():
    if "nc" not in _CACHED:
        _CACHED["nc"] = _build()
    return _CACHED["nc"]


def _make_in_maps(state, x, src):
    state = np.ascontiguousarray(np.asarray(state, dtype=np.float32))
    x = np.ascontiguousarray(np.asarray(x, dtype=np.float32))
    src = np.asarray(src)
    keep = (src != 0).astype(np.float16)
    return [{"state": state[b], "x": x[b], "keep": keep[b]} for b in range(B)]


def run_bass(state, x, src, trace=False, **trace_kwargs):
    from concourse.bass_utils import run_bass_kernel_spmd

    nc = get_nc()
    in_maps = _make_in_maps(state, x, src)
    res = run_bass_kernel_spmd(
        nc, in_maps, core_ids=list(range(B)), trace=trace, **trace_kwargs
    )
    out = np.stack([res.results[b]["out"] for b in range(B)]).astype(np.float32)
    return out, res


def kernel(state, x, src, **kwargs):
    out, _ = run_bass(state, x, src, trace=False)
    return out


if __name__ == "__main__":
    rng = np.random.default_rng(0)
    st = rng.standard_normal((B, S, D), dtype=np.float32)
    xx = rng.standard_normal((B, S, D), dtype=np.float32)
    sr = rng.integers(0, 5, size=(B, S))
    o = kernel(state=st, x=xx, src=sr)
    print(o.shape, o.dtype, np.abs(o).max())
